# revision 1
# baseline (speedup 1.0000x reference)
"""TRN2 Bass kernel for nn_CNNDSTv2_batch: out = mobius16(zeta16(M[:,0]) * zeta16(M[:,1])).

Math: the 16-bit superset-zeta factorizes as Z = A8 @ X @ A8^T on the 256x256
view X[hi_byte, lo_byte]; A8 = [[A7, A7], [0, A7]] block-triangular, so each
8-bit stage is 3 accumulating 128x128 matmuls reusing one stationary. Each
two-sided transform runs as [stage, transpose, stage] and yields the transposed
result; chaining zeta -> multiply -> mobius lands back in natural layout.

Precision (v2 change vs the hi/lo-everywhere baseline): the Mobius side
amplifies errors injected on q = z0*z1 by ~||q||/||m|| ~ 1e2, so q is split
into f32r hi/lo (~23 bits -> ~1e-5 final). Errors on y (half-zeta) and
u (half-mobius) are amplified only ~10-15x / ~10x because the remaining
positive-sum stages average them, so those intermediates are SINGLE f32r
planes (~11 bits -> ~2e-3 final, tolerance is 2e-2). Raw-input f32r rounding
commutes with the positive-sum conjunction and is negligible. Dropping the
y/u splits removes 1/3 of all matmul work and half the transposes.

Perf: walrus's disabled enable-ldw-opt pass is turned on (run_command shim) so
weight loads are separated from matmuls and overlap them (2x matmul issue
rate). It miscompiles nothing we emit: regular f32r matmuls and f32r
transposes are verified bit-correct under it; fp32/bf16 paths are avoided.

Sharding: pure data parallel, batch 512 -> 64 per core across 8 cores.
"""
import sys
import os
import functools

sys.path.insert(0, "/opt/trn_rl_repo")
import numpy as np

BATCH = 512
L = 65536
NCORES = 8
BPC = BATCH // NCORES          # 64 batch elems per core
PAIRS = BPC // 2               # 2 elems per pipeline iteration
G = 3                          # pairs interleaved per pipeline group


def _pc(v):
    return bin(v).count("1")


def _constants():
    k = np.arange(128)
    sup = (k[:, None] & k[None, :]) == k[None, :]          # sup[k,m] = k superset of m
    AT7 = sup.astype(np.float32)                           # lhsT for A7 @ x
    pc = np.array([_pc(i) for i in range(128)])
    sign = (-1.0) ** (pc[:, None] - pc[None, :])
    BT7 = (sup * sign).astype(np.float32)                  # lhsT for B7 @ x
    return AT7, BT7


def _patch_ldw_opt():
    import concourse.bass_utils as bu
    if getattr(bu, "_ldw_opt_patched", False):
        return
    orig = bu.run_command

    def patched(argv, **kw):
        argv = [a.replace("--enable-ldw-opt=false", "--enable-ldw-opt=true")
                for a in argv]
        return orig(argv, **kw)

    bu.run_command = patched
    bu._ldw_opt_patched = True


def _build():
    import concourse.bacc as bacc
    import concourse.tile as tile
    import concourse.mybir as mybir

    _patch_ldw_opt()

    dt = mybir.dt
    F32, F32R = dt.float32, dt.float32r

    nc = bacc.Bacc("TRN2", target_bir_lowering=False, debug=False)

    # HBM layout (host pre-permuted, all DMAs contiguous):
    # Mi[pair, ch, p(=bits14..8), (b, I=bit15, J=bit7, l=bits6..0)]
    Mi = nc.dram_tensor("Mi", [PAIRS, 2, 128, 1024], F32R, kind="ExternalInput").ap()
    # C = [AT7 | BT7 | -BT7] as f32r (exact 0/+-1), IdR = f32r identity
    C = nc.dram_tensor("C", [128, 384], F32R, kind="ExternalInput").ap()
    Id_d = nc.dram_tensor("Id", [128, 128], F32R, kind="ExternalInput").ap()
    # O[pair, p, (I''=bit15, b, J=bit7, l=bits6..0)] - host unscrambles
    O = nc.dram_tensor("O", [PAIRS, 128, 1024], F32, kind="ExternalOutput").ap()

    with tile.TileContext(nc) as tc:
        with tc.tile_pool(name="const", bufs=1) as cp, \
             tc.tile_pool(name="sbuf", bufs=2) as sb, \
             tc.tile_pool(name="psA", bufs=4, space="PSUM") as psA:
            psB = psA
            Ct = cp.tile([128, 384], F32R, tag="C")
            nc.sync.dma_start(Ct[:], C)
            IdR = cp.tile([128, 128], F32R, tag="IdR")
            nc.sync.dma_start(IdR[:], Id_d)
            AT = Ct[:, 0:128]
            BT = Ct[:, 128:256]
            nBT = Ct[:, 256:384]

            def mm(out_ap, lhsT, rhs, start, stop):
                nc.tensor.matmul(out_ap, lhsT, rhs, start=start, stop=stop)

            def stage(dst, M, Mn, s0, s1):
                """dst[:, :512] = M@s0 + Mn@s1 ; dst[:, 512:] = M@s1.
                s0/s1: lists of 1-2 moving APs (f32r planes)."""
                d1 = dst[:, 512:1024]
                for i, a in enumerate(s1):
                    mm(d1, M, a, start=(i == 0), stop=(i == len(s1) - 1))
                d0 = dst[:, 0:512]
                for i, a in enumerate(s0):
                    mm(d0, M, a, start=(i == 0), stop=False)
                for i, a in enumerate(s1):
                    mm(d0, Mn, a, start=False, stop=(i == len(s1) - 1))

            def transpose_plane(dst, src):
                """dst[:, Jd*512 + b*256 + K*128 +: 128] =
                   src[:, K*512 + b*256 + Jd*128 +: 128].T  for Jd,b,K in {0,1}.
                f32r is_transpose (1.5 cyc/row); under ldw-opt the stationary is
                rounded to f32r, a no-op on pre-rounded data. One start/stop
                group per 512-wide PSUM bank."""
                for Jd in (0, 1):
                    k = 0
                    for b in (0, 1):
                        for K in (0, 1):
                            nc.tensor.matmul(
                                dst[:, Jd * 512 + b * 256 + K * 128:][:, :128],
                                src[:, K * 512 + b * 256 + Jd * 128:][:, :128],
                                IdR[:], is_transpose=True,
                                start=(k == 0), stop=(k == 3))
                            k += 1

            # --- software-pipelined emission: G pairs interleaved ---
            st = {}

            def dma_in(pr, c):
                xin = sb.tile([128, 1024], F32R, tag=f"xin{c}", bufs=2 * G - 1,
                              name=f"xin{c}")
                nc.sync.dma_start(xin[:], Mi[pr, c])
                st[pr, c, "x"] = xin

            def preadd_x(pr, c):
                xr = st[pr, c, "x"][:].rearrange("p (b i f) -> p b i f", b=2, i=2)
                sx = sb.tile([128, 512], F32R, tag=f"sx{c}", name=f"sx{c}", bufs=G)
                nc.gpsimd.tensor_add(sx[:], xr[:, :, 0], xr[:, :, 1])
                st[pr, c, "sx"] = sx

            def zeta_s1(pr, c):
                xr = st[pr, c, "x"][:].rearrange("p (b i f) -> p b i f", b=2, i=2)
                y = psA.tile([128, 1024], F32, tag="a", name="y")
                mm(y[:, 512:1024], AT, xr[:, :, 1], start=True, stop=True)
                mm(y[:, 0:512], AT, st[pr, c, "sx"][:], start=True, stop=True)
                st[pr, c, "y"] = y

            def round_y(pr, c):
                """y PSUM f32 -> single f32r plane (the only y rounding)."""
                y = st[pr, c, "y"]
                yf = sb.tile([128, 1024], F32R, tag=f"yf{c}", name=f"yf{c}", bufs=G)
                nc.scalar.copy(yf[:], y[:])
                st[pr, c, "yf"] = yf

            def trans_y(pr, c):
                yT = psB.tile([128, 1024], F32R, tag="a", name="yT")
                transpose_plane(yT[:], st[pr, c, "yf"][:])
                st[pr, c, "T"] = yT

            def copy_T(pr, c):
                yTs = sb.tile([128, 1024], F32R, tag=f"yTs{c}", name=f"yTs{c}",
                              bufs=3)
                nc.vector.tensor_copy(yTs[:], st[pr, c, "T"][:])
                st[pr, c, "Ts"] = yTs

            def preadd_yT(pr, c):
                yTs = st[pr, c, "Ts"]
                sy = sb.tile([128, 512], F32R, tag=f"sy{c}", name=f"sy{c}", bufs=G)
                nc.vector.tensor_add(sy[:], yTs[:, 0:512], yTs[:, 512:1024])
                st[pr, c, "sy"] = sy

            def zeta_s2(pr, c):
                yTs = st[pr, c, "Ts"]
                z = psA.tile([128, 1024], F32, tag="a", name="z")
                mm(z[:, 512:1024], AT, yTs[:, 512:1024], start=True, stop=True)
                mm(z[:, 0:512], AT, st[pr, c, "sy"][:], start=True, stop=True)
                if c == 0:
                    z0s = sb.tile([128, 1024], F32, tag="z0s", name="z0s", bufs=G)
                    nc.scalar.copy(z0s[:], z[:])
                    st[pr, "z0s"] = z0s
                else:
                    t = sb.tile([128, 1024], F32, tag="t", name="t", bufs=3)
                    nc.vector.tensor_mul(t[:], z[:], st[pr, "z0s"][:])
                    st[pr, "t"] = t

            def presub_t(pr):
                # d0 of mobius stage-1 needs B7 @ (q0 - q1): form the
                # difference exactly in f32 BEFORE the hi/lo split
                t = st[pr, "t"]
                sq = sb.tile([128, 512], F32, tag="sq", name="sq")
                nc.vector.tensor_sub(sq[:], t[:, 0:512], t[:, 512:1024])
                st[pr, "sq"] = sq

            def split_q(pr):
                t, sq = st[pr, "t"], st[pr, "sq"]
                qh = sb.tile([128, 1024], F32R, tag="qh", name="qh", bufs=3)
                nc.scalar.copy(qh[:, 0:512], sq[:])
                nc.gpsimd.tensor_copy(qh[:, 512:1024], t[:, 512:1024])
                ql = sb.tile([128, 1024], F32R, tag="ql", name="ql", bufs=3)
                nc.vector.tensor_sub(ql[:, 0:512], sq[:],
                                     qh[:, 0:512].bitcast(F32))
                nc.vector.tensor_sub(ql[:, 512:1024], t[:, 512:1024],
                                     qh[:, 512:1024].bitcast(F32))
                st[pr, "q"] = (qh, ql)

            def mob_s1(pr):
                qh, ql = st[pr, "q"]
                u = psA.tile([128, 1024], F32, tag="a", name="u")
                mm(u[:, 512:1024], BT, qh[:, 512:1024], start=True, stop=False)
                mm(u[:, 512:1024], BT, ql[:, 512:1024], start=False, stop=True)
                mm(u[:, 0:512], BT, qh[:, 0:512], start=True, stop=False)
                mm(u[:, 0:512], BT, ql[:, 0:512], start=False, stop=True)
                st[pr, "u"] = u

            def round_u(pr):
                uf = sb.tile([128, 1024], F32R, tag="uf", name="uf", bufs=G)
                nc.scalar.copy(uf[:], st[pr, "u"][:])
                st[pr, "uf"] = uf

            def trans_u(pr):
                uT = psB.tile([128, 1024], F32R, tag="a", name="uT")
                transpose_plane(uT[:], st[pr, "uf"][:])
                st[pr, "uT"] = uT

            def copy_uT(pr):
                uTs = sb.tile([128, 1024], F32R, tag="uTs", name="uTs", bufs=3)
                nc.vector.tensor_copy(uTs[:], st[pr, "uT"][:])
                st[pr, "uTs"] = uTs

            def presub_uT(pr):
                uTs = st[pr, "uTs"]
                su = sb.tile([128, 512], F32R, tag="su", name="su")
                nc.gpsimd.tensor_sub(su[:], uTs[:, 0:512], uTs[:, 512:1024])
                st[pr, "su"] = su

            def mob_s2(pr):
                uTs = st[pr, "uTs"]
                o = psA.tile([128, 1024], F32, tag="a", name="o")
                mm(o[:, 512:1024], BT, uTs[:, 512:1024], start=True, stop=True)
                mm(o[:, 0:512], BT, st[pr, "su"][:], start=True, stop=True)
                osb = sb.tile([128, 1024], F32, tag="osb", name="osb")
                nc.scalar.copy(osb[:], o[:])
                nc.sync.dma_start(O[pr], osb[:])

            def zeta_wave(prs, c):
                for pr in prs:
                    preadd_x(pr, c)
                for pr in prs:
                    zeta_s1(pr, c)
                for pr in prs:
                    round_y(pr, c)
                for pr in prs:
                    trans_y(pr, c)
                for pr in prs:
                    copy_T(pr, c)
                for pr in prs:
                    preadd_yT(pr, c)
                for pr in prs:
                    zeta_s2(pr, c)

            def mob_head(prs):
                for pr in prs:
                    presub_t(pr)
                for pr in prs:
                    split_q(pr)
                for pr in prs:
                    mob_s1(pr)
                for pr in prs:
                    round_u(pr)

            def mob_tail(prs):
                for pr in prs:
                    trans_u(pr)
                for pr in prs:
                    copy_uT(pr)
                for pr in prs:
                    presub_uT(pr)
                for pr in prs:
                    mob_s2(pr)

            # software-pipelined at group level: group g's zeta waves are
            # emitted between group g-1's mobius head and tail so the PE
            # always has independent work while the q-chain (mul -> presub
            # -> split on ACT/Pool/DVE) completes.
            for pr in range(0, min(G, PAIRS)):
                dma_in(pr, 0)
                dma_in(pr, 1)
            prev = None
            for g in range(0, PAIRS, G):
                prs = range(g, min(g + G, PAIRS))
                for pr in range(g + G, min(g + 2 * G, PAIRS)):
                    dma_in(pr, 0)
                    dma_in(pr, 1)
                zeta_wave(prs, 0)
                if prev is not None:
                    mob_head(prev)
                zeta_wave(prs, 1)
                if prev is not None:
                    mob_tail(prev)
                prev = prs
            mob_head(prev)
            mob_tail(prev)

    nc.compile()
    return nc


@functools.lru_cache(maxsize=1)
def _get_nc():
    return _build()


def _host_in(M):
    """M [512, 2, 65536] f32 -> per-core Mi [PAIRS, 2, 128, 1024] contiguous.
    index16 = I*2^15 + p*2^8 + J*2^7 + l ; f-order (b, I, J, l)."""
    M6 = np.asarray(M, dtype=np.float32).reshape(NCORES, PAIRS, 2, 2, 2, 128, 2, 128)
    #                                      core, pair, b,  ch, I,  p,   J,  l
    Mi = np.ascontiguousarray(M6.transpose(0, 1, 3, 5, 2, 4, 6, 7))
    #                                      core, pair, ch, p, b, I, J, l
    return Mi.reshape(NCORES, PAIRS, 2, 128, 1024)


def _host_out(Os):
    """Os list of [PAIRS, 128, 1024] per core -> [512, 65536, 1, 1].
    o f-layout (I'', b, J, l)."""
    O = np.stack(Os).reshape(NCORES, PAIRS, 128, 2, 2, 2, 128)
    #                         core, pair, p, I, b, J, l
    out = np.ascontiguousarray(O.transpose(0, 1, 4, 3, 2, 5, 6))
    #                                      core, pair, b, I, p, J, l
    return out.reshape(BATCH, L, 1, 1)


def _run(M, trace=False):
    from concourse.bass_utils import run_bass_kernel_spmd
    nc = _get_nc()
    AT7, BT7 = _constants()
    C = np.concatenate([AT7, BT7, -BT7], axis=1)
    Id = np.eye(128, dtype=np.float32)
    Mi = _host_in(M)
    in_maps = [{"Mi": Mi[k], "C": C, "Id": Id} for k in range(NCORES)]
    res = run_bass_kernel_spmd(nc, in_maps, list(range(NCORES)), trace=trace)
    out = _host_out([res.results[k]["O"] for k in range(NCORES)])
    return out, res


def kernel(M):
    try:
        out, _ = _run(M, trace=False)
    except Exception:
        # one retry: a cold first execute has been observed to flake
        # (NRT_EXEC_UNIT_UNRECOVERABLE) and recover on rerun
        out, _ = _run(M, trace=False)
    return out



# revision 2
# speedup vs baseline: 2.0514x; 2.0514x over previous
"""TRN2 Bass kernel for nn_CNNDSTv2_batch: out = mobius16(zeta16(M[:,0]) * zeta16(M[:,1])).

Math: the 16-bit superset-zeta factorizes as Z = A8 @ X @ A8^T on the 256x256
view X[hi_byte, lo_byte]; A8 = [[A7, A7], [0, A7]] block-triangular, so each
8-bit side is A7 (128x128) matmuls plus a preadd (zeta) / +- accumulation
(mobius) for the block bit.

v3 (all-bf16, zero transposes): every PE op is a REGULAR bf16 matmul
(1 cyc/row at any free size). The side swap that used to need PE transposes
is fused into the matmul via DATA-STATIONARY stages: loading the data chunk
as the PE stationary and streaming the constant as the moving operand
computes out = chunk^T @ C — transform + transpose in one instruction.
Chain per batch element:
  zeta A (data-stationary, contracts hi):  yT[l, (J,b,I',p')]   per channel
  zeta B (const-stationary, contracts lo): z [l', (J',b,I',p')]
  q = z0*z1 in f32, split hi/lo bf16
  mob A (data-stationary, contracts lo, J'-bit via +-BT accumulation):
                                           u [p', (I',b,J'',l'')]
  mob B (const-stationary, contracts hi, I'-bit via +-BT accumulation):
                                           m [p'', (I'',b,J'',l'')]

Precision (sim-validated): q and u carried as hi/lo bf16 pairs (~16 bits);
y single bf16; inputs and outputs bf16. Expected L2 ~4e-3 (tolerance 2e-2).
The +-BT accumulation for mobius block bits avoids re-rounding differences
(PSUM accumulates in f32); zeta block bits use preadds (positive sums, no
cancellation, bf16 rounding benign).

bf16 matmuls get bass-split Ldweights (overlapped with matmuls by the PE's
dual pipe natively) — walrus's enable-ldw-opt must stay OFF (it rejects the
pre-split form).

GpSimd (Pool) cannot touch PSUM; DVE tensor_tensor reads at most one PSUM
operand. Engine budget per pair (2 batch elems): PE ~10240 matmul rows,
ACT ~5120 rows, DVE ~4096 rows, Pool ~1024 bf16 rows.

Sharding: pure data parallel, batch 512 -> 64 per core across 8 cores.
"""
import sys
import functools

sys.path.insert(0, "/opt/trn_rl_repo")
import numpy as np
import ml_dtypes

BF = ml_dtypes.bfloat16

BATCH = 512
L = 65536
NCORES = 8
BPC = BATCH // NCORES          # 64 batch elems per core
PAIRS = BPC // 2               # 2 elems per pipeline iteration
G = 4                          # pairs interleaved per pipeline group


def _pc(v):
    return bin(v).count("1")


def _constants():
    k = np.arange(128)
    sup = (k[:, None] & k[None, :]) == k[None, :]          # sup[k,m] = k superset of m
    AT7 = sup.astype(np.float32)                           # A7^T: works as lhsT and rhs
    pc = np.array([_pc(i) for i in range(128)])
    sign = (-1.0) ** (pc[:, None] - pc[None, :])
    BT7 = (sup * sign).astype(np.float32)                  # B7^T
    return AT7, BT7


def _build():
    import concourse.bacc as bacc
    import concourse.tile as tile
    import concourse.mybir as mybir

    dt = mybir.dt
    F32, BF16 = dt.float32, dt.bfloat16

    nc = bacc.Bacc("TRN2", target_bir_lowering=False, debug=False)

    # HBM layout (host pre-permuted + pre-cast bf16, all DMAs contiguous):
    # Mi[pair, ch, p(=bits14..8), (I=bit15, b, J=bit7, l=bits6..0)]
    Mi = nc.dram_tensor("Mi", [PAIRS, 2, 128, 1024], BF16, kind="ExternalInput").ap()
    # C = [AT7 | BT7 | -BT7] bf16 (exact 0/+-1)
    C = nc.dram_tensor("C", [128, 384], BF16, kind="ExternalInput").ap()
    # O[pair, p''(=bits14..8), (I''=bit15, b, J''=bit7, l''=bits6..0)] bf16
    O = nc.dram_tensor("O", [PAIRS, 128, 1024], BF16, kind="ExternalOutput").ap()

    with tile.TileContext(nc) as tc:
        with tc.tile_pool(name="const", bufs=1) as cp, \
             tc.tile_pool(name="sbuf", bufs=2) as sb, \
             tc.tile_pool(name="ps", bufs=4, space="PSUM") as ps:
            Ct = cp.tile([128, 384], BF16, tag="C")
            nc.sync.dma_start(Ct[:], C)
            AT = Ct[:, 0:128]
            BT = Ct[:, 128:256]
            nBT = Ct[:, 256:384]

            def mm(out_ap, lhsT, rhs, start, stop):
                nc.tensor.matmul(out_ap, lhsT, rhs, start=start, stop=stop)

            st = {}

            def dma_in(pr, c):
                xin = sb.tile([128, 1024], BF16, tag=f"xin{c}", bufs=2 * G,
                              name=f"xin{c}")
                nc.sync.dma_start(xin[:], Mi[pr, c])
                st[pr, c, "x"] = xin

            def sx_op(pr, c):
                # I-bit preadd for zeta A: sx[p,(b,J,l)] = x[I=0] + x[I=1]
                xin = st[pr, c, "x"]
                sx = sb.tile([128, 512], BF16, tag=f"sx{c}", bufs=G,
                             name=f"sx{c}")
                nc.gpsimd.tensor_add(sx[:], xin[:, 0:512], xin[:, 512:1024])
                st[pr, c, "sx"] = sx

            def zetaA(pr, c):
                # data-stationary: yT[l, (J,b,I',p')] = chunk^T @ AT
                xin, sx = st[pr, c, "x"], st[pr, c, "sx"]
                yT = ps.tile([128, 1024], F32, tag="a", name="yT")
                for b in (0, 1):
                    for J in (0, 1):
                        src = b * 256 + J * 128
                        dst = J * 512 + b * 256
                        mm(yT[:, dst:dst + 128], sx[:, src:src + 128],
                           AT, start=True, stop=True)
                        mm(yT[:, dst + 128:dst + 256],
                           xin[:, 512 + src:512 + src + 128],
                           AT, start=True, stop=True)
                st[pr, c, "yT"] = yT

            def yf1_op(pr, c):
                # J=1 half of yT rounded to bf16 (stage-B moving operand d1)
                yT = st[pr, c, "yT"]
                yf1 = sb.tile([128, 512], BF16, tag=f"yf1{c}", bufs=G,
                              name=f"yf1{c}")
                nc.scalar.copy(yf1[:], yT[:, 512:1024])
                st[pr, c, "yf1"] = yf1

            def syT_op(pr, c):
                # J-bit preadd for zeta B: syT = yT[J=0] (PSUM f32) + yf1
                yT = st[pr, c, "yT"]
                syT = sb.tile([128, 512], BF16, tag=f"syT{c}", bufs=G,
                              name=f"syT{c}")
                nc.vector.tensor_add(syT[:], yT[:, 0:512], st[pr, c, "yf1"][:])
                st[pr, c, "syT"] = syT

            def zetaB(pr, c):
                # const-stationary: z[l', (J',b,I',p')]
                z = ps.tile([128, 1024], F32, tag="a", name="z")
                mm(z[:, 512:1024], AT, st[pr, c, "yf1"][:], start=True, stop=True)
                mm(z[:, 0:512], AT, st[pr, c, "syT"][:], start=True, stop=True)
                st[pr, c, "z"] = z

            def z0s_op(pr):
                # channel-0 conjunct to SBUF f32 (full precision for the mul)
                z0s = sb.tile([128, 1024], F32, tag="z0s", bufs=G, name="z0s")
                nc.scalar.copy(z0s[:], st[pr, 0, "z"][:])
                st[pr, "z0s"] = z0s

            def t_op(pr):
                # q^T = z0 * z1 in f32 (one PSUM operand max)
                t = sb.tile([128, 1024], F32, tag="t", bufs=3, name="t")
                nc.vector.tensor_mul(t[:], st[pr, 1, "z"][:], st[pr, "z0s"][:])
                st[pr, "t"] = t

            def qh_op(pr):
                qh = sb.tile([128, 1024], BF16, tag="qh", bufs=G, name="qh")
                nc.scalar.copy(qh[:], st[pr, "t"][:])
                st[pr, "qh"] = qh

            def ql_op(pr):
                ql = sb.tile([128, 1024], BF16, tag="ql", bufs=G, name="ql")
                nc.vector.tensor_sub(ql[:], st[pr, "t"][:], st[pr, "qh"][:])
                st[pr, "ql"] = ql

            def mobA(pr):
                # data-stationary, contracts lo(l') with J'-bit via +-BT:
                # u[p', (I',b,J'',l'')]
                qh, ql = st[pr, "qh"], st[pr, "ql"]
                u = ps.tile([128, 1024], F32, tag="a", name="u")
                for b in (0, 1):
                    for Ip in (0, 1):
                        q0 = b * 256 + Ip * 128            # J'=0 chunk
                        q1 = 512 + q0                      # J'=1 chunk
                        d0 = Ip * 512 + b * 256            # J''=0 block
                        d1 = d0 + 128                      # J''=1 block
                        mm(u[:, d0:d0 + 128], qh[:, q0:q0 + 128], BT,
                           start=True, stop=False)
                        mm(u[:, d0:d0 + 128], ql[:, q0:q0 + 128], BT,
                           start=False, stop=False)
                        mm(u[:, d0:d0 + 128], qh[:, q1:q1 + 128], nBT,
                           start=False, stop=False)
                        mm(u[:, d0:d0 + 128], ql[:, q1:q1 + 128], nBT,
                           start=False, stop=True)
                        mm(u[:, d1:d1 + 128], ql[:, q1:q1 + 128], BT,
                           start=True, stop=False)
                        mm(u[:, d1:d1 + 128], qh[:, q1:q1 + 128], BT,
                           start=False, stop=True)
                st[pr, "u"] = u

            def uh_op(pr):
                uh = sb.tile([128, 1024], BF16, tag="uh", bufs=G, name="uh")
                nc.scalar.copy(uh[:], st[pr, "u"][:])
                st[pr, "uh"] = uh

            def ul_op(pr):
                ul = sb.tile([128, 1024], BF16, tag="ul", bufs=G, name="ul")
                nc.vector.tensor_sub(ul[:], st[pr, "u"][:], st[pr, "uh"][:])
                st[pr, "ul"] = ul

            def mobB(pr):
                # const-stationary, contracts hi(p') with I'-bit via +-BT:
                # m[p'', (I'',b,J'',l'')]
                uh, ul = st[pr, "uh"], st[pr, "ul"]
                o = ps.tile([128, 1024], F32, tag="a", name="o")
                mm(o[:, 512:1024], BT, uh[:, 512:1024], start=True, stop=False)
                mm(o[:, 512:1024], BT, ul[:, 512:1024], start=False, stop=True)
                mm(o[:, 0:512], BT, uh[:, 0:512], start=True, stop=False)
                mm(o[:, 0:512], BT, ul[:, 0:512], start=False, stop=False)
                mm(o[:, 0:512], nBT, uh[:, 512:1024], start=False, stop=False)
                mm(o[:, 0:512], nBT, ul[:, 512:1024], start=False, stop=True)
                st[pr, "o"] = o

            def osb_op(pr):
                osb = sb.tile([128, 1024], BF16, tag="osb", bufs=3, name="osb")
                nc.scalar.copy(osb[:], st[pr, "o"][:])
                nc.sync.dma_start(O[pr], osb[:])

            def zeta_wave(prs, c):
                for pr in prs:
                    sx_op(pr, c)
                for pr in prs:
                    zetaA(pr, c)
                for pr in prs:
                    yf1_op(pr, c)
                for pr in prs:
                    syT_op(pr, c)
                for pr in prs:
                    zetaB(pr, c)
                if c == 0:
                    for pr in prs:
                        z0s_op(pr)

            def mob_head(prs):
                for pr in prs:
                    t_op(pr)
                for pr in prs:
                    qh_op(pr)
                for pr in prs:
                    ql_op(pr)

            def mob_tail(prs):
                for pr in prs:
                    mobA(pr)
                for pr in prs:
                    uh_op(pr)
                for pr in prs:
                    ul_op(pr)
                for pr in prs:
                    mobB(pr)
                for pr in prs:
                    osb_op(pr)

            # software-pipelined at group level: group g's zeta waves are
            # emitted between group g-1's mobius head and tail so the PE
            # always has independent work while the q-chain completes.
            for pr in range(0, min(G, PAIRS)):
                dma_in(pr, 0)
                dma_in(pr, 1)
            prev = None
            for g in range(0, PAIRS, G):
                prs = range(g, min(g + G, PAIRS))
                for pr in range(g + G, min(g + 2 * G, PAIRS)):
                    dma_in(pr, 0)
                    dma_in(pr, 1)
                zeta_wave(prs, 0)
                if prev is not None:
                    mob_head(prev)
                zeta_wave(prs, 1)
                if prev is not None:
                    mob_tail(prev)
                prev = prs
            mob_head(prev)
            mob_tail(prev)

    nc.compile()
    return nc


@functools.lru_cache(maxsize=1)
def _get_nc():
    return _build()


def _host_in(M):
    """M [512, 2, 65536] f32 -> per-core Mi [PAIRS, 2, 128, 1024] bf16.
    index16 = I*2^15 + p*2^8 + J*2^7 + l ; f-order (I, b, J, l)."""
    M6 = np.asarray(M, dtype=np.float32).reshape(
        NCORES, PAIRS, 2, 2, 2, 128, 2, 128)
    #   core, pair, b, ch, I, p, J, l
    Mi = np.ascontiguousarray(M6.transpose(0, 1, 3, 5, 4, 2, 6, 7).astype(BF))
    #   core, pair, ch, p, I, b, J, l
    return Mi.reshape(NCORES, PAIRS, 2, 128, 1024)


def _host_out(Os):
    """Os list of [PAIRS, 128, 1024] bf16 per core -> [512, 65536, 1, 1] f32.
    o f-layout (I'', b, J'', l'')."""
    O = np.stack(Os).astype(np.float32).reshape(
        NCORES, PAIRS, 128, 2, 2, 2, 128)
    #   core, pair, p, I, b, J, l
    out = np.ascontiguousarray(O.transpose(0, 1, 4, 3, 2, 5, 6))
    #   core, pair, b, I, p, J, l
    return out.reshape(BATCH, L, 1, 1)


def _run(M, trace=False):
    from concourse.bass_utils import run_bass_kernel_spmd
    nc = _get_nc()
    AT7, BT7 = _constants()
    C = np.concatenate([AT7, BT7, -BT7], axis=1).astype(BF)
    Mi = _host_in(M)
    in_maps = [{"Mi": Mi[k], "C": C} for k in range(NCORES)]
    res = run_bass_kernel_spmd(nc, in_maps, list(range(NCORES)), trace=trace)
    out = _host_out([res.results[k]["O"] for k in range(NCORES)])
    return out, res


def kernel(M):
    try:
        out, _ = _run(M, trace=False)
    except Exception:
        # one retry: a cold first execute has been observed to flake
        # (NRT_EXEC_UNIT_UNRECOVERABLE) and recover on rerun
        out, _ = _run(M, trace=False)
    return out
